# revision 16
# baseline (speedup 1.0000x reference)
"""Trainium2 Bass kernel for nn_LLAConv2d: per-sample 1x1 conv with mixed kernels.

Math: out[b,o,h,w] = sum_i K[b,o,i] * x[b,i,h,w],  K[b] = sum_e alpha[b,e]*ke[e]
i.e. a per-sample 64x64 matmul over 160*160=25600 pixels. Memory-bound.

Strategy (8 cores, data-parallel over batch, 4 samples/core):
  - Pack 2 samples per matmul: block-diagonal lhsT [128,128] built on device
    (zeros + two 64x64 blocks K[b]^T), rhs = x tile [128(2*64 chans), N].
  - Kernel mixing on device: broadcast alpha via ones-matmul, then 8
    tensor_scalar MACs per 64x64 block from a pre-transposed embed table.
  - Matmuls run as float32r (full-rate fp32) with N=512 per PSUM bank,
    5 per 2560-column tile; PSUM->SBUF copy on vector engine; 1.28MB DMAs.
"""

import os
import sys

sys.path.insert(0, "/opt/trn_rl_repo")

import numpy as np

import concourse.bacc as bacc
import concourse.bass as bass
import concourse.mybir as mybir
import concourse.tile as tile
from concourse import bass_utils

F32 = mybir.dt.float32
F32R = mybir.dt.float32r

N_CORES = 8
B, E, CIN, COUT, H, W = 32, 8, 64, 64, 160, 160
PIX = H * W                     # 25600
BPC = B // N_CORES              # 4 samples per core
NPAIR = BPC // 2                # 2 sample-pairs per core
TILE_N = 2560                   # pixels per DMA tile (1.28 MB per [128, 2560] f32)
NT = PIX // TILE_N              # 10 tiles per pair
MM_N = 512                      # matmul free dim (one PSUM bank of fp32)
KPT = TILE_N // MM_N            # 5 matmuls per tile

# knob: use float32r (full-rate fp32 matmul) vs exact fp32 (4x slower on PE)
MM_DTYPE = F32 if os.environ.get("KERNEL_MM_FP32") else F32R

LAST_RESULTS = None  # test.py reads exec_time_ns / trace info from here


def _build_bass():
    nc = bacc.Bacc(trn_type="TRN2", target_bir_lowering=False, debug=False)

    # x is declared float32r (same bits as f32) so the DMA into SBUF counts as
    # "rounded to FP32r" for the matmul verifier; numpy side stays float32.
    x_d = nc.dram_tensor("x", [NPAIR, 128, PIX], MM_DTYPE, kind="ExternalInput").ap()
    ket_d = nc.dram_tensor("ket", [64, E * 64], F32, kind="ExternalInput").ap()
    al_d = nc.dram_tensor("alpha", [1, BPC * E], F32, kind="ExternalInput").ap()
    out_d = nc.dram_tensor("out", [NPAIR, 128, PIX], F32, kind="ExternalOutput").ap()

    with tile.TileContext(nc) as tc:
        with (
            tc.tile_pool(name="wpool", bufs=1) as wpool,
            tc.tile_pool(name="tmppool", bufs=2) as tmppool,
            tc.tile_pool(name="xpool", bufs=4) as xpool,
            tc.tile_pool(name="opool", bufs=4) as opool,
            tc.tile_pool(name="ppool", bufs=7, space="PSUM") as ppool,
            tc.tile_pool(name="papool", bufs=1, space="PSUM") as papool,
        ):
            # --- setup: load embed table (duplicated on both partition halves)
            ket_sb = wpool.tile([128, E * 64], F32, tag="ket_sb")
            nc.sync.dma_start(out=ket_sb[0:64, :], in_=ket_d)
            nc.sync.dma_start(out=ket_sb[64:128, :], in_=ket_d)

            al_sb = wpool.tile([1, BPC * E], F32, tag="al_sb")
            nc.sync.dma_start(out=al_sb, in_=al_d)

            # broadcast alpha to all 128 partitions: ones(128,1) @ alpha(1,32)
            ones = wpool.tile([1, 128], F32, tag="ones")
            nc.vector.memset(ones, 1.0)
            al_ps = papool.tile([128, BPC * E], F32, tag="al_ps")
            nc.tensor.matmul(al_ps, ones, al_sb, start=True, stop=True)
            al_bc = wpool.tile([128, BPC * E], F32, tag="al_bc")
            nc.vector.tensor_copy(al_bc, al_ps)

            # --- build block-diagonal lhsT tiles, one per sample-pair
            # lhsT[i, o] = K[b]^T in the diagonal 64x64 blocks, zero elsewhere
            lhsT = []
            for p in range(NPAIR):
                t = wpool.tile([128, 128], F32, tag=f"mix{p}", name=f"mix{p}")
                nc.vector.memset(t, 0.0)
                for h in range(2):
                    s = 2 * p + h          # sample index within shard
                    pr = slice(64 * h, 64 * h + 64)
                    blk = t[pr, 64 * h : 64 * h + 64]
                    nc.vector.tensor_scalar_mul(
                        blk, ket_sb[pr, 0:64], al_bc[pr, s * E : s * E + 1]
                    )
                    for e in range(1, E):
                        tmp = tmppool.tile([128, 64], F32, tag="tmp")
                        nc.vector.tensor_scalar_mul(
                            tmp[pr, :],
                            ket_sb[pr, e * 64 : e * 64 + 64],
                            al_bc[pr, s * E + e : s * E + e + 1],
                        )
                        nc.vector.tensor_add(blk, blk, tmp[pr, :])
                # final copy rounds the mixed weights to the matmul dtype
                tr = wpool.tile([128, 128], MM_DTYPE, tag=f"lhsT{p}", name=f"lhsT{p}")
                nc.vector.tensor_copy(tr, t)
                lhsT.append(tr)

            # --- main loop: load x tile, 5 matmuls, copy psum, store
            for j in range(NT):
                for p in range(NPAIR):
                    c0 = j * TILE_N
                    xt = xpool.tile([128, TILE_N], MM_DTYPE, tag="xt")
                    nc.sync.dma_start(out=xt, in_=x_d[p, :, c0 : c0 + TILE_N])
                    ot = opool.tile([128, TILE_N], F32, tag="ot")
                    for k in range(KPT):
                        pt = ppool.tile([128, MM_N], F32, tag="pt")
                        nc.tensor.matmul(
                            pt,
                            lhsT[p],
                            xt[:, k * MM_N : (k + 1) * MM_N],
                            start=True,
                            stop=True,
                        )
                        nc.vector.tensor_copy(ot[:, k * MM_N : (k + 1) * MM_N], pt)
                    nc.sync.dma_start(out=out_d[p, :, c0 : c0 + TILE_N], in_=ot)

    nc.compile()
    return nc


def kernel(x, alpha, kernel_embed):
    global LAST_RESULTS
    x = np.ascontiguousarray(x, dtype=np.float32)
    alpha = np.ascontiguousarray(alpha, dtype=np.float32)
    ke = np.ascontiguousarray(kernel_embed, dtype=np.float32)[:, :, :, 0, 0]
    # ket[i, e*64+o] = ke[e, o, i]
    ket = np.ascontiguousarray(np.transpose(ke, (2, 0, 1)).reshape(64, E * 64))

    in_maps = []
    for c in range(N_CORES):
        xs = x[c * BPC : (c + 1) * BPC].reshape(NPAIR, 128, PIX)
        als = alpha[c * BPC : (c + 1) * BPC].reshape(1, BPC * E)
        in_maps.append(
            {
                "x": np.ascontiguousarray(xs),
                "alpha": np.ascontiguousarray(als),
                "ket": ket,
            }
        )

    nc = _build_bass()
    res = bass_utils.run_bass_kernel_spmd(
        nc,
        in_maps,
        core_ids=list(range(N_CORES)),
        trace=bool(os.environ.get("KERNEL_TRACE")),
    )
    LAST_RESULTS = res

    out = np.empty((B, COUT, H, W), dtype=np.float32)
    for c in range(N_CORES):
        out[c * BPC : (c + 1) * BPC] = res.results[c]["out"].reshape(BPC, COUT, H, W)
    return out


def bench(x, alpha, kernel_embed, iters=30):
    """Estimate device exec time: build the same sharded PJRT executable as
    run_bass_via_pjrt, pre-place inputs on device, fire `iters` async calls
    (fresh donated zero-output buffers staged off-clock), block once."""
    import time

    import jax
    from jax.sharding import Mesh, NamedSharding, PartitionSpec
    from jax.experimental.shard_map import shard_map

    from concourse import bass2jax

    x = np.ascontiguousarray(x, dtype=np.float32)
    alpha = np.ascontiguousarray(alpha, dtype=np.float32)
    ke = np.ascontiguousarray(kernel_embed, dtype=np.float32)[:, :, :, 0, 0]
    ket = np.ascontiguousarray(np.transpose(ke, (2, 0, 1)).reshape(64, E * 64))

    in_maps = []
    for c in range(N_CORES):
        xs = x[c * BPC : (c + 1) * BPC].reshape(NPAIR, 128, PIX)
        als = alpha[c * BPC : (c + 1) * BPC].reshape(1, BPC * E)
        in_maps.append(
            {"x": np.ascontiguousarray(xs), "alpha": np.ascontiguousarray(als), "ket": ket}
        )

    nc = _build_bass()
    bass2jax.install_neuronx_cc_hook()

    import concourse.mybir as mybir_

    in_names, out_names, out_avals, zero_outs = [], [], [], []
    for alloc in nc.m.functions[0].allocations:
        if not isinstance(alloc, mybir_.MemoryLocationSet):
            continue
        name = alloc.memorylocations[0].name
        pid = nc.partition_id_tensor.name if nc.partition_id_tensor else None
        if alloc.kind == "ExternalInput":
            if name != pid:
                in_names.append(name)
        elif alloc.kind == "ExternalOutput":
            out_names.append(name)
            dtype = mybir_.dt.np(alloc.dtype)
            out_avals.append(
                jax.core.ShapedArray(tuple(alloc.tensor_shape), dtype)
            )
            zero_outs.append(np.zeros(tuple(alloc.tensor_shape), dtype))
    n_params = len(in_names)
    all_names = in_names + out_names
    if nc.partition_id_tensor is not None:
        all_names = all_names + [nc.partition_id_tensor.name]

    def _body(*args):
        operands = list(args)
        if nc.partition_id_tensor is not None:
            operands.append(bass2jax.partition_id_tensor())
        return tuple(
            bass2jax._bass_exec_p.bind(
                *operands,
                out_avals=tuple(out_avals),
                in_names=tuple(all_names),
                out_names=tuple(out_names),
                lowering_input_output_aliases=(),
                sim_require_finite=True,
                sim_require_nnan=True,
                nc=nc,
            )
        )

    devices = jax.devices()[:N_CORES]
    mesh = Mesh(np.asarray(devices), ("core",))
    spec = PartitionSpec("core")
    donate = tuple(range(n_params, n_params + len(out_names)))
    fn = jax.jit(
        shard_map(
            _body,
            mesh=mesh,
            in_specs=(spec,) * (n_params + len(out_names)),
            out_specs=(spec,) * len(out_names),
            check_rep=False,
        ),
        donate_argnums=donate,
        keep_unused=True,
    )
    sh = NamedSharding(mesh, spec)
    concat_in = [
        jax.device_put(
            np.concatenate([in_maps[c][n] for c in range(N_CORES)], axis=0), sh
        )
        for n in in_names
    ]
    # one warmup (compiles), then stage per-iter donated zero buffers
    warm_zeros = [
        jax.device_put(np.zeros((N_CORES * z.shape[0], *z.shape[1:]), z.dtype), sh)
        for z in zero_outs
    ]
    jax.block_until_ready(fn(*concat_in, *warm_zeros))

    zsets = []
    for _ in range(iters):
        zs = [
            jax.device_put(np.zeros((N_CORES * z.shape[0], *z.shape[1:]), z.dtype), sh)
            for z in zero_outs
        ]
        zsets.append(zs)
    jax.block_until_ready(zsets)

    # serial timing (per-call, includes one dispatch each)
    per_call = []
    for zs in zsets[: iters // 2]:
        t0 = time.perf_counter()
        jax.block_until_ready(fn(*concat_in, *zs))
        per_call.append(time.perf_counter() - t0)

    # pipelined timing (amortizes dispatch)
    rest = zsets[iters // 2 :]
    t0 = time.perf_counter()
    outs = [fn(*concat_in, *zs) for zs in rest]
    jax.block_until_ready(outs)
    pipelined = (time.perf_counter() - t0) / max(1, len(rest))

    return {
        "serial_min_ns": min(per_call) * 1e9,
        "serial_med_ns": sorted(per_call)[len(per_call) // 2] * 1e9,
        "pipelined_ns": pipelined * 1e9,
    }
